# revision 48
# baseline (speedup 1.0000x reference)
"""Bass/Tile Trainium2 kernel for dense causal multi-head attention.

Problem: x[2,2048,1024] -> qkv (w_qkv [3072,1024]) -> 16-head causal
attention -> out proj (w_proj [1024,1024], b_proj) -> [2,2048,1024].

Sharding over 8 NeuronCores: data-parallel over batch (2) x
tensor-parallel over heads (4 groups of 4 heads). Each core computes its
qkv slice, causal attention for its 4 heads (2 pairs of 2), and a
partial output projection over its 256 head-dim columns. The all-reduce
after proj is realized host-side at gather time (sum of 4 partials per
batch) together with the bias add.

v2 design (vs the fp32r baseline):
  * bf16 everywhere on the matmul paths (inputs converted host-side):
    halves all DMA, removes the fp32r small-free-dim penalty so causal
    trimming works at 128-column granularity, and unlocks the 2x/4x DVE
    modes for masks/normalize.
  * V is produced directly in [token, feature] layout by a transposed
    GEMM (stationary = x^T tile, moving = both pairs' 256 v-weight
    columns) - no PE transposes, no vt scratch.
  * One set of PSUM pools lives for the whole kernel (s:4 banks,
    av0/av1+pbc: 2, mm/vt/proj/warm: 2 = 8 banks) so there are no
    pool-boundary barriers and the PE clock governor never sees a gap.
  * pair-0 qkv streams chunk-by-chunk inside pair-0's attention
    (drained right before the chunk that needs it), overlapping the
    input DMA; pair-1 qkv fills pair-0's exp shadows; the projection
    fills pair-1's.
  * Causal work trimmed to the 128-tile diagonal: S/exp/AV only touch
    query columns >= the k-tile start, one shared [128,256] triangular
    mask handles the diagonal blocks.
"""

import sys
from contextlib import ExitStack

if "/opt/trn_rl_repo" not in sys.path:
    sys.path.insert(0, "/opt/trn_rl_repo")

import numpy as np

import concourse.bass as bass
import concourse.tile as tile
from concourse import bacc, mybir
from concourse.bass_utils import run_bass_kernel_spmd

F32 = mybir.dt.float32
BF16 = mybir.dt.bfloat16
AF = mybir.ActivationFunctionType

B, N, C = 2, 2048, 1024
H_TOT, D = 16, 64
NCORES = 8
HPC = H_TOT // (NCORES // B)  # heads per core = 4
HD = HPC * D                  # 256 per-core head-dim columns
CT = C // 128                 # 8 contraction tiles
NT = N // 128                 # 16 seq tiles
QCH = N // 512                # 4 query chunks of 512
SCALE = float(D) ** -0.5
WARM_N = 60                   # PE clock warm-up matmuls (fills DMA head)


class Ctx:
    """Shared build state."""
    pass


def _qk_chunk_units(g, col0, dest, nch, evict_eng):
    """Units for one 128-col j-tile of the q/k GEMM, query chunk nch:
    4 units of 2 accumulating matmuls + 1 evict."""
    nc = g.nc
    ns = slice(nch * 512, (nch + 1) * 512)
    cell = {}
    units = []

    def mk_mm(cts):
        def u():
            if "ps" not in cell:
                cell["ps"] = g.mm_ps.tile([128, 512], F32, tag="mm",
                                          bufs=2, name="gps")
            for ct in cts:
                nc.tensor.matmul(
                    cell["ps"][:],
                    g.wqk[:, ct, col0:col0 + 128],
                    g.xt[:, ct, ns],
                    start=(ct == 0), stop=(ct == CT - 1),
                )
        return u

    def mk_evict():
        def u():
            if evict_eng == "act":
                nc.scalar.activation(dest[:, ns], cell["ps"][:], AF.Copy)
            else:
                nc.vector.tensor_copy(dest[:, ns], cell["ps"][:])
        return u

    units.append(mk_mm([0, 1]))
    units.append(mk_mm([2, 3]))
    units.append(mk_mm([4, 5]))
    units.append(mk_mm([6, 7]))
    units.append(mk_evict())
    return units


def _vt_units(g, nt, hps=(0, 1)):
    """Units for V^T of seq tile nt for head pairs `hps`: out[token, feat]
    = sum_ct xT_tile^T-stationary @ wv-moving; evict interleaves the
    even/odd head halves into v_sb around the ones columns. bf16 matmuls
    run 1 cyc/row at any width, so a single-pair (128-col) split costs
    the same total cycles — used to shift pair-1's V work into phase 2b."""
    nc = g.nc
    w = 128 * len(hps)
    c0 = 128 * hps[0]
    cell = {}
    units = []

    def mk_mm(cts):
        def u():
            if "ps" not in cell:
                cell["ps"] = g.mm_ps.tile([128, w], F32, tag="mm",
                                          bufs=2, name="vps")
            for ct in cts:
                nc.tensor.matmul(
                    cell["ps"][:],
                    g.xt[:, ct, nt * 128:(nt + 1) * 128],
                    g.wv[:, ct, c0:c0 + w],
                    start=(ct == 0), stop=(ct == CT - 1),
                )
        return u

    def mk_evict():
        def u():
            for k, hp in enumerate(hps):
                vd = g.v_sb[hp][:, nt, :]
                nc.vector.tensor_copy(
                    vd.rearrange("p (b c) -> p b c", b=2)[:, :, 0:64],
                    cell["ps"][:, 128 * k:128 * k + 128]
                    .rearrange("p (b c) -> p b c", b=2))
        return u

    units.append(mk_mm([0, 1, 2, 3]))
    units.append(mk_mm([4, 5, 6, 7]))
    units.append(mk_evict())
    return units


def _proj_units(g, qc):
    """Filler units for the output projection of seq tiles in chunk qc."""
    nc = g.nc
    units = []
    for nt in range(4 * qc, 4 * qc + 4):
        cell = {}

        def mk_mm(ht, nt=nt, cell=cell):
            def u():
                if "ps" not in cell:
                    cell["ps"] = [g.mm_ps.tile([128, 512], F32, tag="mm",
                                               bufs=2, name="pso")
                                  for _ in range(2)]
                for cok in range(2):
                    nc.tensor.matmul(
                        cell["ps"][cok][:],
                        g.yT[:, ht, nt * 128:(nt + 1) * 128],
                        g.wp[ht][:, cok * 512:(cok + 1) * 512],
                        start=(ht == 0), stop=(ht == 1),
                    )
            return u

        def mk_out(cok, nt=nt, cell=cell):
            def u():
                ot = g.o_pool.tile([128, 512], BF16, tag="ot", name="ot")
                nc.vector.tensor_copy(ot[:], cell["ps"][cok][:])
                nc.sync.dma_start(
                    g.out_r[nt, :, cok * 512:(cok + 1) * 512], ot[:])
            return u

        units.extend([mk_mm(0), mk_mm(1), mk_out(0), mk_out(1)])
    return units


def _attention_pair(g, hp, fillers, avsb_eng, pre_chunk_cb=None,
                    chunk_cb=None, barriers=None):
    """Causal attention for head pair hp, popping filler units into the
    exp (ScalarE) shadow of each k-tile step. Work is trimmed at
    128-column granularity along the causal diagonal."""
    nc = g.nc
    q_t, k_t = g.q_t[hp], g.k_t[hp]
    # per chunk: nkt k-tile steps (+1 extra pop weight on each of the two
    # short diagonal steps) + 2 normalize steps
    total_steps = sum(4 * (qc + 1) + 2 + 2 for qc in range(QCH))
    state = {"fi": 0, "step": 0}

    def pop(nsteps, reserve=0):
        state["step"] += nsteps
        left = total_steps - state["step"]
        avail = len(fillers) - state["fi"] - reserve
        want = avail if left <= 0 else -(-avail // (left + 1)) * nsteps
        for _ in range(min(want, avail)):
            fillers[state["fi"]]()
            state["fi"] += 1

    def pop_until(idx):
        # barrier: everything before `idx` in the filler list must have
        # been issued (used for weights/V a later chunk depends on)
        while state["fi"] < min(idx, len(fillers)):
            fillers[state["fi"]]()
            state["fi"] += 1

    def make_pts(qc, kt, qs):
        # both heads' S^T tiles into one 2-bank PSUM tile -> a single
        # exp (amortizes the ScalarE fixed overhead). Diagonal k-tiles
        # only compute query columns >= the k-tile start.
        rs = 0 if kt < 4 * qc else 128 * (kt - 4 * qc)
        ps = g.s_ps.tile([128, 2, 512], F32, tag="s", bufs=2, name="pss")
        for po in range(2):
            o = 64 * po
            nc.tensor.matmul(
                ps[:, po, rs:],
                k_t[o:o + 64, kt * 128:(kt + 1) * 128],
                q_t[o:o + 64, qs][:, rs:],
                start=True, stop=True,
            )
        ptb = g.p_pool.tile([128, 2, 512], BF16, tag="pt", name="pt")
        nc.scalar.activation(ptb[:, :, rs:], ps[:, :, rs:], AF.Exp,
                             scale=SCALE)
        if kt >= 4 * qc:
            # triangular mask on the diagonal 128-block; DVE runs these
            # at 4x (all-bf16 SBUF operands), and with the normalize muls
            # on GpSimd its queue is shallow enough to never stall AV.
            for po in range(2):
                nc.vector.tensor_mul(ptb[:, po, rs:rs + 128],
                                     ptb[:, po, rs:rs + 128],
                                     g.tri[:, 128:256])
        return rs, [ptb[:, 0, :], ptb[:, 1, :]]

    for qc in range(QCH):
        if barriers is not None and qc in barriers:
            pop_until(barriers[qc])
        if pre_chunk_cb is not None:
            pre_chunk_cb(qc)
        nkt = 4 * (qc + 1)
        qs = slice(qc * 512, (qc + 1) * 512)
        pav = [g.av_ps.tile([65, 512], F32, tag=f"av{po}", bufs=1,
                            name=f"pav{po}") for po in range(2)]
        # software pipeline, depth 3: AV for k-tile kt-3 issues after S
        # for kt, so the AV weight-load never waits on a fresh exp+mask.
        pend = []
        for kt in range(nkt + 3):
            s_rs = 0
            if kt < nkt:
                s_rs = 0 if kt < 4 * qc else 128 * (kt - 4 * qc)
                pend.append((kt,) + make_pts(qc, kt, qs))
            if kt >= 3:
                akt, rs, pts = pend.pop(0)
                for po in range(2):
                    nc.tensor.matmul(
                        pav[po][:, rs:],
                        g.v_sb[hp][:, akt, 65 * po:65 * po + 65],
                        pts[po][:, rs:],
                        start=(akt == 0), stop=(akt == nkt - 1),
                    )
                # short diagonal steps leave PE slack: pop extra fillers
                pop(2 if s_rs >= 256 else 1)
        # last chunk's normalize feeds the projection drain directly:
        # keep it on low-latency DVE; elsewhere the muls go to the idle
        # GpSimd so DVE stays free to recycle filler PSUM promptly.
        # last chunk's normalize feeds the projection drain directly: run
        # both heads' chains interleaved on DVE and emit the muls
        # nt-by-nt so the first projection tile starts ~2us earlier.
        # Elsewhere the muls go to the idle GpSimd so DVE stays free to
        # recycle filler PSUM promptly (all-DVE mid-phase measured worse).
        tail = (qc == QCH - 1)
        if tail:
            avs, bcs = [], []
            for po in range(2):
                av = g.r_pool.tile([65, 512], BF16, tag="avsb",
                                   name="avsb")
                if avsb_eng == "act":
                    nc.scalar.activation(av[:], pav[po][:], AF.Copy)
                else:
                    nc.vector.tensor_copy(av[:], pav[po][:])
                avs.append(av)
            pop(1)
            for po in range(2):
                pbc = g.av_ps.tile([65, 512], F32, tag=f"av{po}", bufs=1,
                                   name="pbc")
                nc.tensor.matmul(pbc[0:64, :], g.ones64[64:65, :],
                                 avs[po][64:65, :], start=True, stop=True)
                bc = g.bc_pool.tile([64, 512], F32, tag="bc", name="bc")
                nc.vector.reciprocal_approx_fast(bc[:], pbc[0:64, :])
                bcs.append(bc)
            pop(1)
            for j in range(4):
                js = slice(128 * j, 128 * (j + 1))
                for po in range(2):
                    nc.vector.tensor_mul(
                        g.yT[64 * po:64 * po + 64, hp, qs][:, js],
                        avs[po][0:64, js], bcs[po][:, js])
            pop(2)
        else:
            for po in range(2):
                # evict the accumulator to SBUF (frees the PSUM slot),
                # then normalize rows 0-63 by broadcast(1/row64) via a
                # rank-1 PE broadcast + fast reciprocal.
                av = g.r_pool.tile([65, 512], BF16, tag="avsb",
                                   name="avsb")
                if avsb_eng == "act":
                    nc.scalar.activation(av[:], pav[po][:], AF.Copy)
                else:
                    nc.vector.tensor_copy(av[:], pav[po][:])
                pbc = g.av_ps.tile([65, 512], F32, tag=f"av{po}", bufs=1,
                                   name="pbc")
                nc.tensor.matmul(pbc[0:64, :], g.ones64[64:65, :],
                                 av[64:65, :], start=True, stop=True)
                bc = g.bc_pool.tile([64, 512], F32, tag="bc", name="bc")
                nc.vector.reciprocal_approx_fast(bc[:], pbc[0:64, :])
                for j in range(4):
                    js = slice(128 * j, 128 * (j + 1))
                    nc.gpsimd.tensor_mul(
                        g.yT[64 * po:64 * po + 64, hp, qs][:, js],
                        av[0:64, js], bc[:, js])
                pop(2)
        if chunk_cb is not None:
            chunk_cb(qc)
    while state["fi"] < len(fillers):
        fillers[state["fi"]]()
        state["fi"] += 1


def build_nc():
    nc = bacc.Bacc("TRN2", target_bir_lowering=False, debug=False)
    xT = nc.dram_tensor("xT", [C, N], BF16, kind="ExternalInput").ap()
    # cols: [q01 | k01 | q23 | k23 | v0123(256)]
    wqkvT = nc.dram_tensor("wqkvT", [C, 768], BF16, kind="ExternalInput").ap()
    wpT = nc.dram_tensor("wpT", [HD, C], BF16, kind="ExternalInput").ap()
    out = nc.dram_tensor("out", [N, C], BF16, kind="ExternalOutput").ap()

    xT_p = xT.rearrange("(ct p) n -> p ct n", p=128)
    w_p = wqkvT.rearrange("(ct p) j -> p ct j", p=128)
    wqk_p = w_p[:, :, 0:512]
    wv_p = w_p[:, :, 512:768]
    wp_r = wpT.rearrange("(ht p) co -> ht p co", p=128)

    g = Ctx()
    g.nc = nc
    g.out_r = out.rearrange("(nt p) co -> nt p co", p=128)

    with tile.TileContext(nc) as tc, ExitStack() as ctx:
        const = ctx.enter_context(tc.tile_pool(name="const", bufs=1))
        qkv_pool = ctx.enter_context(tc.tile_pool(name="qkv", bufs=1))
        yT_pool = ctx.enter_context(tc.tile_pool(name="yT", bufs=1))
        v_pool = ctx.enter_context(tc.tile_pool(name="v", bufs=1))
        x_pool = ctx.enter_context(tc.tile_pool(name="x", bufs=1))
        w_pool = ctx.enter_context(tc.tile_pool(name="w", bufs=1))
        g.p_pool = ctx.enter_context(tc.tile_pool(name="p", bufs=8))
        g.r_pool = ctx.enter_context(tc.tile_pool(name="avsb", bufs=3))
        g.bc_pool = ctx.enter_context(tc.tile_pool(name="bcast", bufs=2))
        g.o_pool = ctx.enter_context(tc.tile_pool(name="o", bufs=4))
        g.s_ps = ctx.enter_context(
            tc.tile_pool(name="sps", bufs=2, space="PSUM"))
        g.av_ps = ctx.enter_context(
            tc.tile_pool(name="avps", bufs=1, space="PSUM"))
        g.mm_ps = ctx.enter_context(
            tc.tile_pool(name="mmps", bufs=2, space="PSUM"))

        wrm = const.tile([128, 128], BF16, tag="wrm")
        nc.vector.memset(wrm[:], 0.125)
        g.wrm = wrm
        ones64 = const.tile([128, 64], BF16, tag="ones64")
        nc.vector.memset(ones64[:], 1.0)
        g.ones64 = ones64
        # [zeros(128) | lower-triangle(128)] causal mask for diagonal
        # blocks: tri[p, 128+c] = 1 iff c >= p.
        g.tri = const.tile([128, 256], BF16, tag="tri")
        nc.gpsimd.memset(g.tri[:], 1.0)
        nc.gpsimd.affine_select(
            out=g.tri[:], in_=g.tri[:], compare_op=mybir.AluOpType.is_ge,
            fill=0.0, base=-128, channel_multiplier=-1, pattern=[[1, 256]],
        )

        # persistent activations / weights (all bf16)
        g.q_t = [qkv_pool.tile([128, N], BF16, tag=f"q{hp}", name=f"qT{hp}")
                 for hp in range(2)]
        g.k_t = [qkv_pool.tile([128, N], BF16, tag=f"k{hp}", name=f"kT{hp}")
                 for hp in range(2)]
        g.yT = yT_pool.tile([128, 2, N], BF16, tag="yT")
        # V per pair: [token-partition, nt, 130] = [v_even |1| v_odd |1]
        g.v_sb = [v_pool.tile([128, NT, 130], BF16, tag=f"v{hp}",
                              name=f"v{hp}") for hp in range(2)]
        for hp in range(2):
            nc.vector.memset(g.v_sb[hp][:, :, 64], 1.0)
            nc.vector.memset(g.v_sb[hp][:, :, 129], 1.0)
        g.xt = x_pool.tile([128, CT, N], BF16, tag="x", name="xt")
        g.wqk = w_pool.tile([128, CT, 512], BF16, tag="wqk", name="wqk")
        g.wv = w_pool.tile([128, CT, 256], BF16, tag="wv", name="wv")
        g.wp = [w_pool.tile([128, C], BF16, tag=f"wp{ht}", name=f"wp{ht}")
                for ht in range(2)]

        # input DMAs, ordered by first use (sync queues preserve order)
        nc.sync.dma_start(g.wqk[:, :, 0:128], wqk_p[:, :, 0:128])
        nc.sync.dma_start(g.xt[:, 0:4, 0:512], xT_p[:, 0:4, 0:512])
        nc.sync.dma_start(g.xt[:, 4:8, 0:512], xT_p[:, 4:8, 0:512])
        nc.sync.dma_start(g.wqk[:, :, 128:256], wqk_p[:, :, 128:256])
        nc.sync.dma_start(g.wv[:], wv_p)
        nc.sync.dma_start(g.xt[:, :, 512:1024], xT_p[:, :, 512:1024])
        nc.sync.dma_start(g.wqk[:, :, 256:512], wqk_p[:, :, 256:512])
        nc.sync.dma_start(g.xt[:, :, 1024:1536], xT_p[:, :, 1024:1536])
        nc.sync.dma_start(g.xt[:, :, 1536:2048], xT_p[:, :, 1536:2048])
        for ht in range(2):
            nc.sync.dma_start(g.wp[ht][:], wp_r[ht])

        # PE clock warm-up: dummy matmuls cover the input-DMA head so the
        # clock governor is at full rate when the real stream starts.
        wps = g.mm_ps.tile([128, 512], F32, tag="mm", bufs=2, name="wps")
        for _ in range(WARM_N):
            nc.tensor.matmul(wps[:, 0:128], wrm[:], wrm[:],
                             start=True, stop=True)

        # ---- pair-0 attention ----
        # Per-chunk inline drain: pair-0 q/k GEMM for that chunk (+ V for
        # chunk 0). V GEMMs for chunks 1-3 and pair-1's first-half q/k
        # GEMM pop as fillers in the exp shadow, with barriers forcing
        # each chunk's V to be issued before the chunk that consumes it.
        def drain0(qc):
            units = []
            units += _qk_chunk_units(g, 0, g.q_t[0], qc, "dve")
            units += _qk_chunk_units(g, 128, g.k_t[0], qc, "dve")
            if qc == 0:
                for nt in range(4):
                    units += _vt_units(g, nt)
            for u in units:
                u()

        f1 = []
        b1 = {}
        for nt in range(4, 8):
            f1 += _vt_units(g, nt)
        b1[1] = len(f1)
        # chunks 2-3: only pair-0's V half here; pair-1's half becomes
        # early phase-2b filler (that phase otherwise starves for PE work)
        for nch in range(2, QCH):
            for nt in range(4 * nch, 4 * nch + 4):
                f1 += _vt_units(g, nt, hps=(0,))
            b1[nch] = len(f1)
        for nch in range(2):
            f1 += _qk_chunk_units(g, 256, g.q_t[1], nch, "act")
            f1 += _qk_chunk_units(g, 384, g.k_t[1], nch, "act")
        _attention_pair(g, 0, f1, "act", pre_chunk_cb=drain0, barriers=b1)

        # ---- pair-1 attention; its second-half q/k and V GEMMs fill the
        # early chunks (before any projection work exists), projection
        # fills the rest ----
        f2 = []
        b2 = {}
        for nch in range(2, QCH):
            for nt in range(4 * nch, 4 * nch + 4):
                f2 += _vt_units(g, nt, hps=(1,))
            # evicts on DVE: ScalarE is the pacing engine in this phase
            f2 += _qk_chunk_units(g, 256, g.q_t[1], nch, "dve")
            f2 += _qk_chunk_units(g, 384, g.k_t[1], nch, "dve")
            b2[nch] = len(f2)

        def chunk_cb(qc):
            f2.extend(_proj_units(g, qc))

        _attention_pair(g, 1, f2, "dve", chunk_cb=chunk_cb, barriers=b2)

    nc.compile()
    return nc


_NC = None


def _get_nc():
    global _NC
    if _NC is None:
        _NC = build_nc()
    return _NC


def make_in_maps(x, w_qkv, w_proj):
    import ml_dtypes
    bf = ml_dtypes.bfloat16
    x = np.asarray(x, dtype=np.float32)
    w_qkv = np.asarray(w_qkv, dtype=np.float32)
    w_proj = np.asarray(w_proj, dtype=np.float32)
    xT = [np.ascontiguousarray(x[b].T.astype(bf)) for b in range(B)]
    in_maps = []
    for c in range(NCORES):
        b, grp = divmod(c, NCORES // B)
        # wqkT cols: [q01 | k01 | q23 | k23] (128 each), wvT: [h0..h3]
        qk_rows = []
        for hp in range(2):
            for s in range(2):  # q, k blocks of w_qkv
                base = s * C + grp * HD + hp * 2 * D
                qk_rows.append(np.arange(base, base + 2 * D))
        qk_rows.append(np.arange(2 * C + grp * HD, 2 * C + (grp + 1) * HD))
        rows = np.concatenate(qk_rows)
        wqkvT = np.ascontiguousarray(w_qkv[rows, :].T.astype(bf))
        wpT = np.ascontiguousarray(
            w_proj[:, grp * HD:(grp + 1) * HD].T.astype(bf))
        in_maps.append({"xT": xT[b], "wqkvT": wqkvT, "wpT": wpT})
    return in_maps


def assemble(results, b_proj):
    b_proj = np.asarray(b_proj, dtype=np.float32)
    out = np.zeros((B, N, C), dtype=np.float32)
    for c in range(NCORES):
        b = c // (NCORES // B)
        out[b] += results[c]["out"].astype(np.float32)
    out += b_proj[None, None, :]
    return out


def kernel(x, w_qkv, w_proj, b_proj):
    nc = _get_nc()
    in_maps = make_in_maps(x, w_qkv, w_proj)
    res = run_bass_kernel_spmd(nc, in_maps, core_ids=list(range(NCORES)))
    return assemble(res.results, b_proj)
